# revision 1
# baseline (speedup 1.0000x reference)
"""GraphGym GeneralConv (GCN-style, add-aggr, symmetric norm) on 8 Trainium2
NeuronCores via Bass/Tile.

Math (matches the reference exactly, up to fp reassociation):
    deg[i]  = 1 + #{e : row[e] == i}
    dis     = deg ** -0.5
    h       = x @ W
    out[i]  = dis[i] * ( sum_{e : col[e] == i} dis[row[e]] * h[row[e]]
                         + dis[i] * h[i] )          # self-loop

Distribution: destination-node sharding.  Core k owns dest nodes
[k*SHARD, (k+1)*SHARD); every edge is routed to the core owning its dest.
Every core redundantly computes the full h = x @ W (x is replicated,
transposed on the host so no on-device transpose is needed), writes it to a
DRAM scratch `h_perm` in a permuted tile-major row layout (so the write is
a single contiguous DMA per slice), then gathers its edges' source rows
edge-major with the SWDGE dma_gather instruction (int16 indices wrapped
over 16 partitions, source windowed into 4 chunks of <=32767 rows),
scatter-adds them into per-dest-block PSUM accumulators with
selection-matrix matmuls on the PE (sel[p, d] = (col_local[slot p] == d) *
dis[row[slot p]], built by one fused DVE tensor_scalar per 128-slot tile),
adds the self-loop term with a diagonal matmul over indirectly-gathered own
rows, scales by dis[dest], and writes its shard as one contiguous DMA.

The host does integer-only preprocessing: degree histogram, edge bucketing
by (core, dest-block, source-chunk), fixed-size slot layout, index/col
packing.  The slot layout is input-independent (fixed run length L_RUN per
bucket), so the Bass program is compiled once and cached; bucket overflow
(a few hundred edges for random graphs) is corrected exactly on the host.
"""

import math

import numpy as np

# ----------------------------------------------------------------------------
# configuration
# ----------------------------------------------------------------------------

N_NODES = 100000
DIM = 64
N_CORES = 8

P = 128  # partitions


class Cfg:
    def __init__(self, n_nodes, dim, n_cores, slots_per_run,
                 blocks_per_group, chunk_slices=2, subcall_runs=None):
        self.N = n_nodes
        self.DIM = dim
        self.NC = n_cores
        # 128-aligned dest shards; the last core's shard may be smaller
        self.NBLK = math.ceil(n_nodes / (n_cores * P))   # dest blocks / core
        self.SHARD = self.NBLK * P               # padded shard size
        # h-permutation slice == shard, uniform J so the per-core self-row
        # window is one affine AP at a register offset
        self.SLICE = self.SHARD
        self.NS = n_cores
        self.J = [self.NBLK] * n_cores
        self.row_base = np.arange(n_cores + 1) * self.SHARD
        self.H_ROWS = int(self.row_base[-1])
        assert self.SHARD * (n_cores - 1) < n_nodes <= self.H_ROWS
        self.CH_SL = chunk_slices                # slices per gather chunk
        self.NCH = math.ceil(self.NS / chunk_slices)
        self.crow = [int(self.row_base[min(c * chunk_slices, self.NS)])
                     for c in range(self.NCH + 1)]
        for c in range(self.NCH):
            assert self.crow[c + 1] - self.crow[c] <= 32767
        self.L_RUN = int(slots_per_run)          # slots per (block,chunk) run
        assert self.L_RUN % 64 == 0
        self.NBG = blocks_per_group              # dest blocks per psum group
        assert self.NBLK % blocks_per_group == 0
        self.NGRP = self.NBLK // blocks_per_group
        self.TOT = self.NBLK * self.NCH * self.L_RUN   # slots per core
        assert self.TOT % P == 0
        self.NTILES = self.TOT // P
        self.SR = subcall_runs or blocks_per_group  # runs per dma_gather call
        assert self.NBG % self.SR == 0
        assert (self.SR * self.L_RUN) % P == 0
        self.CALL_SLOTS = self.SR * self.L_RUN
        self.NCALLS = self.NGRP * self.NCH * (self.NBG // self.SR)
        self.IDXW = self.TOT // 16

    def run_subtiles(self, g, c, b_):
        """K-subtiles of run (g, c, b_): [(abs_slot0, K), ...]."""
        out = []
        base = ((g * self.NCH + c) * self.NBG + b_) * self.L_RUN
        s = base
        end = s + self.L_RUN
        while s < end:
            if s % P:
                k = P - s % P
            else:
                k = min(P, end - s)
            out.append((s, k))
            s += k
        return out


CFG = Cfg(N_NODES, DIM, N_CORES, slots_per_run=640,
          blocks_per_group=7, subcall_runs=7)


def rho(cfg, n):
    """node id -> permuted h_perm row (vectorized)."""
    s = n // cfg.SLICE
    m = n - s * cfg.SLICE
    J = np.asarray(cfg.J)[s]
    return cfg.row_base[s] + (m % P) * J + m // P


# ----------------------------------------------------------------------------
# host preprocessing
# ----------------------------------------------------------------------------

def host_prep(cfg, x, weight, edge_index):
    x = np.asarray(x, dtype=np.float32)
    weight = np.asarray(weight, dtype=np.float32)
    ei = np.asarray(edge_index)
    row = ei[0].astype(np.int64)
    col = ei[1].astype(np.int64)

    # deg counts outgoing (row) edges plus the implicit self-loop
    deg = (np.bincount(row, minlength=cfg.N) + 1).astype(np.float32)

    k = np.minimum(col // cfg.SHARD, cfg.NC - 1)
    blk = (col % cfg.SHARD) // P
    col_local = (col % cfg.SHARD) % P
    g = blk // cfg.NBG
    b_ = blk % cfg.NBG
    s = row // cfg.SLICE
    c = np.minimum(s // cfg.CH_SL, cfg.NCH - 1)
    prow = rho(cfg, row)
    idxrel = prow - np.asarray(cfg.crow)[c]

    run_in_core = (g * cfg.NCH + c) * cfg.NBG + b_
    key = k * (cfg.NBLK * cfg.NCH) + run_in_core
    order = np.argsort(key, kind="stable")
    key_s = key[order]
    counts = np.bincount(key_s, minlength=cfg.NC * cfg.NBLK * cfg.NCH)
    starts = np.concatenate([[0], np.cumsum(counts)])
    pos = np.arange(key_s.size) - starts[key_s]

    ok = pos < cfg.L_RUN
    slot = run_in_core[order] * cfg.L_RUN + pos   # slot within core
    kk = k[order]

    idx_flat = np.zeros((cfg.NC, cfg.TOT), dtype=np.int16)
    colv = np.full((cfg.NC, cfg.TOT), -1.0, dtype=np.float32)
    degrow = np.ones((cfg.NC, cfg.TOT), dtype=np.float32)

    o = order[ok]
    idx_flat[kk[ok], slot[ok]] = idxrel[o].astype(np.int16)
    colv[kk[ok], slot[ok]] = col_local[o].astype(np.float32)
    degrow[kk[ok], slot[ok]] = deg[row[o]]

    # overflow edges -> host correction (expected: a handful)
    ov = order[~ok]

    def pack(a):
        return np.ascontiguousarray(
            a.reshape(cfg.NC, cfg.NTILES, P).transpose(0, 2, 1))

    # per-call 16-partition wrapping of indices, replicated to 128 partitions
    idxw = idx_flat.reshape(cfg.NC, cfg.NCALLS, cfg.CALL_SLOTS // 16, 16)
    idxw = idxw.transpose(0, 3, 1, 2).reshape(cfg.NC, 16, cfg.IDXW)
    idxv_p = np.ascontiguousarray(np.tile(idxw, (1, 8, 1)))

    colv_p = pack(colv)
    degrow_p = pack(degrow)

    # dest-side degrees [NC, 128, NBLK]
    degdest = np.ones((cfg.NC, cfg.NBLK * P), dtype=np.float32)
    ids = np.arange(cfg.SHARD)
    for core in range(cfg.NC):
        nd = min(cfg.SHARD, cfg.N - core * cfg.SHARD)
        degdest[core, :nd] = deg[core * cfg.SHARD + ids[:nd]]
    degdest = np.ascontiguousarray(
        degdest.reshape(cfg.NC, cfg.NBLK, P).transpose(0, 2, 1))

    # per-block self rows: permuted h row of each dest node [NC, 128, NBLK]
    selfidx = np.zeros((cfg.NC, cfg.NBLK * P), dtype=np.int32)
    for core in range(cfg.NC):
        nd = min(cfg.SHARD, cfg.N - core * cfg.SHARD)
        selfidx[core, :nd] = rho(cfg, core * cfg.SHARD + ids[:nd])
    selfidx = np.ascontiguousarray(
        selfidx.reshape(cfg.NC, cfg.NBLK, P).transpose(0, 2, 1))

    xt = np.ascontiguousarray(x.T)
    iota = np.broadcast_to(np.arange(P, dtype=np.float32), (P, P)).copy()
    partidx = np.arange(P, dtype=np.float32).reshape(P, 1).copy()

    in_maps = []
    for core in range(cfg.NC):
        in_maps.append({
            "xt": xt,
            "w": weight,
            "iota": iota,
            "partidx": partidx,
            "colv": colv_p[core],
            "degrow": degrow_p[core],
            "degdest": degdest[core],
            "selfidx": selfidx[core],
            "idx": idxv_p[core],
        })

    # host correction for overflowed edges
    corr = None
    if ov.size:
        r, cdst = row[ov], col[ov]
        hsrc = x[r] @ weight
        m = hsrc * (deg[r] ** -0.5 * deg[cdst] ** -0.5)[:, None]
        corr = np.zeros((cfg.N, cfg.DIM), dtype=np.float32)
        np.add.at(corr, cdst, m)
    return in_maps, corr


def unshard(cfg, outs, corr):
    out = np.empty((cfg.N, cfg.DIM), dtype=np.float32)
    for core in range(cfg.NC):
        o = outs[core]["outp"].reshape(P, cfg.NBLK, cfg.DIM)
        o = o.transpose(1, 0, 2).reshape(cfg.NBLK * P, cfg.DIM)
        nd = min(cfg.SHARD, cfg.N - core * cfg.SHARD)
        out[core * cfg.SHARD:core * cfg.SHARD + nd] = o[:nd]
    if corr is not None:
        out += corr
    return out


# ----------------------------------------------------------------------------
# device program
# ----------------------------------------------------------------------------

_PROG_CACHE = {}


def build_program(cfg, reps=1, phases="12", queue_map=None):
    import contextlib

    import concourse.bass as bass
    import concourse.tile as tile
    from concourse import bacc, mybir

    f32 = mybir.dt.float32
    nc = bacc.Bacc("TRN2", target_bir_lowering=False, debug=False,
                   num_devices=cfg.NC, num_swdge_queues=4)

    xt = nc.dram_tensor("xt", [cfg.DIM, cfg.N], f32, kind="ExternalInput")
    w = nc.dram_tensor("w", [cfg.DIM, cfg.DIM], f32, kind="ExternalInput")
    iota = nc.dram_tensor("iota", [P, P], f32, kind="ExternalInput")
    partidx = nc.dram_tensor("partidx", [P, 1], f32, kind="ExternalInput")
    selfidx = nc.dram_tensor("selfidx", [P, cfg.NBLK], mybir.dt.int32,
                             kind="ExternalInput")
    colv = nc.dram_tensor("colv", [P, cfg.NTILES], f32, kind="ExternalInput")
    degrow = nc.dram_tensor("degrow", [P, cfg.NTILES], f32,
                            kind="ExternalInput")
    degdest = nc.dram_tensor("degdest", [P, cfg.NBLK], f32,
                             kind="ExternalInput")
    idx = nc.dram_tensor("idx", [P, cfg.IDXW], mybir.dt.int16,
                         kind="ExternalInput")
    outp = nc.dram_tensor("outp", [P, cfg.NBLK * cfg.DIM], f32,
                          kind="ExternalOutput")
    h_perm = nc.dram_tensor("h_perm", [cfg.H_ROWS, cfg.DIM], f32)

    PSB = 8  # h tiles batched per psum bank
    nc._gather_insts = []

    with tile.TileContext(nc) as tc:
      with (tc.For_i(0, reps, 1) if reps > 1 else contextlib.nullcontext()):
        # ---------------- phase 1: h = x @ W, permuted layout ----------------
        if "1" in phases:
            with tc.tile_pool(name="p1s", bufs=2) as sp, \
                 tc.tile_pool(name="p1c", bufs=1) as cp, \
                 tc.tile_pool(name="p1p", bufs=4, space="PSUM") as pp:
                w_sb = cp.tile([cfg.DIM, cfg.DIM], f32)
                nc.sync.dma_start(out=w_sb[:], in_=w[:])
                for s in range(cfg.NS):
                    J = cfg.J[s]
                    n0 = s * cfg.SLICE
                    nn = min(cfg.SLICE, cfg.N - n0)
                    xs = sp.tile([cfg.DIM, cfg.SLICE], f32, tag="xs")
                    nc.sync.dma_start(out=xs[:, :nn], in_=xt[:, n0:n0 + nn])
                    if nn < P * J:
                        # pad the tail tile so every psum row is written
                        nc.vector.memset(xs[:, nn:P * J], 0)
                    hs = sp.tile([P, cfg.J[0] * cfg.DIM], f32, tag="hs")
                    for m in range(math.ceil(J / PSB)):
                        j0 = m * PSB
                        jn = min(PSB, J - j0)
                        ps = pp.tile([P, PSB * cfg.DIM], f32)
                        for j8 in range(jn):
                            j = j0 + j8
                            nc.tensor.matmul(
                                out=ps[:, j8 * cfg.DIM:(j8 + 1) * cfg.DIM],
                                lhsT=xs[:, j * P:(j + 1) * P],
                                rhs=w_sb[:],
                                start=True, stop=True)
                        nc.vector.tensor_copy(
                            out=hs[:, j0 * cfg.DIM:(j0 + jn) * cfg.DIM],
                            in_=ps[:, :jn * cfg.DIM])
                    dst = h_perm[cfg.row_base[s]:cfg.row_base[s] + P * J, :]
                    dst = dst.rearrange("(p j) d -> p (j d)", p=P)
                    nc.sync.dma_start(out=dst, in_=hs[:, :J * cfg.DIM])

        # ---------------- phase 2: indirect gather + PE scatter-add ---------
        if set("2GM") & set(phases):
            mode = ("full" if "2" in phases else
                    "gather" if "G" in phases else "mm")
            with tc.tile_pool(name="p2c", bufs=1) as cp, \
                 tc.tile_pool(name="p2g", bufs=3) as gp, \
                 tc.tile_pool(name="p2sel", bufs=6) as selp, \
                 tc.tile_pool(name="p2p", bufs=2, space="PSUM") as pp:
                iota_sb = cp.tile([P, P], f32)
                nc.sync.dma_start(out=iota_sb[:], in_=iota[:])
                pidx_sb = cp.tile([P, 1], f32)
                nc.sync.dma_start(out=pidx_sb[:], in_=partidx[:])
                colv_sb = cp.tile([P, cfg.NTILES], f32)
                nc.sync.dma_start(out=colv_sb[:], in_=colv[:])
                selv_sb = cp.tile([P, cfg.NTILES], f32)
                nc.sync.dma_start(out=selv_sb[:], in_=degrow[:])
                # dis[row] = 1/sqrt(deg[row])
                nc.scalar.sqrt(out=selv_sb[:], in_=selv_sb[:])
                nc.vector.reciprocal(out=selv_sb[:], in_=selv_sb[:])
                disd_sb = cp.tile([P, cfg.NBLK], f32)
                nc.sync.dma_start(out=disd_sb[:], in_=degdest[:])
                nc.scalar.sqrt(out=disd_sb[:], in_=disd_sb[:])
                nc.vector.reciprocal(out=disd_sb[:], in_=disd_sb[:])
                sidx_sb = cp.tile([P, cfg.NBLK], mybir.dt.int32)
                nc.sync.dma_start(out=sidx_sb[:], in_=selfidx[:])
                idx_sb = cp.tile([P, cfg.IDXW], mybir.dt.int16)
                nc.sync.dma_start(out=idx_sb[:], in_=idx[:])
                out_sb = cp.tile([P, cfg.NBLK * cfg.DIM], f32)

                if mode == "mm":
                    shared_sel = cp.tile([P, P], f32)
                    nc.vector.tensor_scalar(
                        out=shared_sel[:], in0=iota_sb[:],
                        scalar1=colv_sb[:, 0:1], scalar2=selv_sb[:, 0:1],
                        op0=mybir.AluOpType.is_equal,
                        op1=mybir.AluOpType.mult)

                gbufs = {}
                CW = cfg.CALL_SLOTS // 16   # idx columns per call
                CT = cfg.CALL_SLOTS // P    # slot tiles per call

                def get_gbuf(T):
                    j = T // CT
                    if j not in gbufs:
                        c = (j // (cfg.NBG // cfg.SR)) % cfg.NCH
                        gb = gp.tile([P, CT, cfg.DIM], f32, tag="gbuf")
                        gi = nc.gpsimd.dma_gather(
                            out_ap=gb[:],
                            in_ap=h_perm[cfg.crow[c]:cfg.crow[c + 1], :],
                            idxs_ap=idx_sb[:, j * CW:(j + 1) * CW],
                            num_idxs=cfg.CALL_SLOTS,
                            num_idxs_reg=cfg.CALL_SLOTS,
                            elem_size=cfg.DIM,
                            single_packet=False,
                            queue_num=(queue_map or {}).get(j, 0),
                        )
                        nc._gather_insts.append((j, gi.ins.name))
                        gbufs[j] = gb
                        if mode == "gather":
                            nc.vector.tensor_copy(out=out_sb[:, :cfg.DIM],
                                                  in_=gb[:, 0, :])
                    return gbufs[j], T % CT

                bank_w = 2048 // (4 * cfg.DIM)  # blocks per psum bank
                for g in range(cfg.NGRP):
                    if mode == "gather":
                        for c in range(cfg.NCH):
                            for b_ in range(cfg.NBG):
                                for (s0, kk) in cfg.run_subtiles(g, c, b_):
                                    get_gbuf(s0 // P)
                        continue
                    ps = pp.tile([P, cfg.NBG * cfg.DIM], f32)
                    for c in range(cfg.NCH):
                      for b_ in range(cfg.NBG):
                        for si, (s0, kk) in enumerate(cfg.run_subtiles(g, c, b_)):
                            gb, tloc = get_gbuf(s0 // P)
                            T = s0 // P
                            p0 = s0 % P
                            if mode == "mm":
                                sel = shared_sel
                            else:
                                sel = selp.tile([P, P], f32)
                                nc.vector.tensor_scalar(
                                    out=sel[p0:p0 + kk, :],
                                    in0=iota_sb[p0:p0 + kk, :],
                                    scalar1=colv_sb[p0:p0 + kk, T:T + 1],
                                    scalar2=selv_sb[p0:p0 + kk, T:T + 1],
                                    op0=mybir.AluOpType.is_equal,
                                    op1=mybir.AluOpType.mult)
                            nc.tensor.matmul(
                                out=ps[:, b_ * cfg.DIM:(b_ + 1) * cfg.DIM],
                                lhsT=sel[p0:p0 + kk, :],
                                rhs=gb[p0:p0 + kk, tloc, :],
                                start=(b_ % bank_w == 0 and c == 0
                                       and si == 0),
                                stop=False, skip_group_check=True)
                    for b_ in range(cfg.NBG):
                        b = g * cfg.NBG + b_
                        # self-loop: psum[:, b_] += diag(dis[dest]) @ h[dest]
                        hself = selp.tile([P, cfg.DIM], f32, tag="hself")
                        nc.gpsimd.indirect_dma_start(
                            out=hself[:], out_offset=None,
                            in_=h_perm[:],
                            in_offset=bass.IndirectOffsetOnAxis(
                                ap=sidx_sb[:, b:b + 1], axis=0))
                        diag = selp.tile([P, P], f32, tag="diag")
                        nc.vector.tensor_scalar(
                            out=diag[:], in0=iota_sb[:],
                            scalar1=pidx_sb[:, 0:1],
                            scalar2=disd_sb[:, b:b + 1],
                            op0=mybir.AluOpType.is_equal,
                            op1=mybir.AluOpType.mult)
                        nc.tensor.matmul(
                            out=ps[:, b_ * cfg.DIM:(b_ + 1) * cfg.DIM],
                            lhsT=diag[:], rhs=hself[:],
                            start=False, stop=True, skip_group_check=True)
                        nc.vector.tensor_scalar_mul(
                            out_sb[:, b * cfg.DIM:(b + 1) * cfg.DIM],
                            ps[:, b_ * cfg.DIM:(b_ + 1) * cfg.DIM],
                            disd_sb[:, b:b + 1])
                nc.sync.dma_start(out=outp[:], in_=out_sb[:])

    nc.compile()
    return nc


def gather_queue_map(nc):
    """call_j -> queue: DMASW lane % 4, except lanes also used by plain
    Pool DMAs (which are implicitly queue 0) are pinned to queue 0."""
    import concourse.mybir as mybir
    from concourse.tile_sem_assignment import PROC_NAME_TO_IDX
    idx2name = {v: k for k, v in PROC_NAME_TO_IDX.items()}
    gather_names = {name for _, name in nc._gather_insts}
    locked = set()
    for name, inst in nc.inst_map.items():
        proc = idx2name.get(getattr(inst, "bass_scheduled_proc", None), "")
        if (proc.startswith("DMASW") and name not in gather_names):
            locked.add(proc)
    qm = {}
    for j, name in nc._gather_insts:
        inst = nc.inst_map[name]
        proc = idx2name[inst.bass_scheduled_proc]
        assert proc.startswith("DMASW")
        qm[j] = 0 if proc in locked else int(proc[5:]) % 4
    return qm


def build_with_queues(cfg, reps=1, phases="12", rotate=False):
    if not rotate:
        return build_program(cfg, reps=reps, phases=phases, queue_map=None)
    qm = {}
    for _ in range(3):
        nc = build_program(cfg, reps=reps, phases=phases, queue_map=qm)
        qm2 = gather_queue_map(nc)
        if qm2 == qm:
            return nc
        qm = qm2
    return build_program(cfg, reps=reps, phases=phases, queue_map=None)


def get_program(cfg):
    key = (cfg.N, cfg.DIM, cfg.NC, cfg.SLICE, cfg.L_RUN, cfg.NBG,
           cfg.CH_SL, cfg.SR)
    if key not in _PROG_CACHE:
        _PROG_CACHE[key] = build_with_queues(cfg)
    return _PROG_CACHE[key]


# ----------------------------------------------------------------------------
# entry point
# ----------------------------------------------------------------------------

def kernel(x, weight, edge_index):
    from concourse.bass_utils import run_bass_kernel_spmd

    cfg = CFG
    in_maps, corr = host_prep(cfg, x, weight, edge_index)
    nc = get_program(cfg)
    res = run_bass_kernel_spmd(nc, in_maps, list(range(cfg.NC)))
    return unshard(cfg, res.results, corr)



# revision 9
# speedup vs baseline: 1.7780x; 1.7780x over previous
"""GraphGym GeneralConv (GCN-style, add-aggr, symmetric norm) on 8 Trainium2
NeuronCores via Bass/Tile.

Math (matches the reference exactly, up to fp reassociation):
    deg[i]  = 1 + #{e : row[e] == i}
    dis     = deg ** -0.5
    h       = x @ W
    out[i]  = sum_{e : col[e] == i} dis[row] * dis[i] * h[row]
              + dis[i]^2 * h[i]                      # self-loop

Distribution: destination-node sharding.  Core k owns dest nodes
[k*SHARD, (k+1)*SHARD); every edge is routed to the core owning its dest.
Every core redundantly computes the full h = x @ W (x is replicated,
transposed on the host), writes it to a DRAM scratch `h_perm` in a
permuted half-slice-major layout (so each write is 128 large contiguous
descriptors), then gathers its edges' source rows edge-major with the
SWDGE dma_gather instruction (int16 indices wrapped over 16 partitions,
source windowed into 4 chunks of <=32767 rows, calls spread round-robin
over all 4 SWDGE queues so all 8 GPSIMD Q7 descriptor-generation cores
run in parallel), scatter-adds them into per-(group,chunk) PSUM
accumulators with selection-matrix matmuls on the PE (sel[p, d] =
(col_local[slot p] == d) * norm[slot p], norm = dis[row]*dis[col] fully
precomputed on the host), and accumulates PSUM into an SBUF output
staging tile per chunk with DVE adds.

The self-loop term needs h rows of the core's own shard only; those are
computed on-chip from a per-core `xt_own` input into a persistent SBUF
tile `hs_own`, and added with one fused DVE scalar_tensor_tensor per
dest block: out += dis[dest]^2 * hs_own.  No indirect DMA is used, so
the 4 SWDGE queues carry nothing but the 56 gather calls.

Phase interleave: chunk c's gathers read only h slices 2c, 2c+1, so the
dense h compute is emitted slice-pair-wise between the chunk gather
batches and overlaps with gather DMA (Tile's range-based dependency
analysis keeps it correct).

The host does integer-only preprocessing: degree histogram, edge
bucketing by (core, dest-block, source-chunk), fixed-size slot layout
(B_RUN slots per bucket, uniform across cores so the single SPMD
program is input-independent and compile-cached), index/col/norm
packing.  Bucket overflow (a few hundred edges for this graph) is
corrected exactly on the host.
"""

import math

import numpy as np

# ----------------------------------------------------------------------------
# configuration
# ----------------------------------------------------------------------------

N_NODES = 100000
DIM = 64
N_CORES = 8

P = 128  # partitions


class Cfg:
    def __init__(self, n_nodes, dim, n_cores, slots_per_run=544,
                 blocks_per_group=7, chunk_slices=2):
        self.N = n_nodes
        self.DIM = dim
        self.NC = n_cores
        # 128-aligned dest shards; the last core's shard may be smaller
        self.NBLK = math.ceil(n_nodes / (n_cores * P))   # dest blocks / core
        self.SHARD = self.NBLK * P               # padded shard size
        self.SLICE = self.SHARD                  # h-permutation slice
        self.NS = n_cores
        self.row_base = np.arange(n_cores + 1) * self.SHARD
        self.H_ROWS = int(self.row_base[-1])
        assert self.SHARD * (n_cores - 1) < n_nodes <= self.H_ROWS
        # half-slice structure for the permuted h layout
        assert self.NBLK % 2 == 0
        self.HALF = self.NBLK // 2               # blocks per half-slice
        self.HS = P * self.HALF                  # rows per half-slice
        # gather source windows (int16 indices limit each to <=32767 rows)
        self.CH_SL = chunk_slices                # slices per gather chunk
        self.NCH = math.ceil(self.NS / chunk_slices)
        self.crow = [int(self.row_base[min(c * chunk_slices, self.NS)])
                     for c in range(self.NCH + 1)]
        for c in range(self.NCH):
            assert self.crow[c + 1] - self.crow[c] <= 32767
        # slot layout: bucket (dest block, chunk) gets B_RUN slots; one
        # gather call per (group, chunk) covers NBG buckets + pad to 128
        self.B_RUN = int(slots_per_run)
        assert self.B_RUN % 16 == 0
        self.NBG = blocks_per_group              # dest blocks per psum group
        assert self.NBLK % blocks_per_group == 0
        self.NGRP = self.NBLK // blocks_per_group
        self.CALL = ((self.NBG * self.B_RUN + P - 1) // P) * P
        self.CT = self.CALL // P                 # gbuf slot tiles per call
        self.CW = self.CALL // 16                # idx columns per call
        self.NCALLS = self.NCH * self.NGRP       # call j = c*NGRP + g
        self.TOT = self.NCALLS * self.CALL       # slots per core
        self.NTILES = self.TOT // P
        self.IDXW = self.NCALLS * self.CW
        # sel data is packed per SUBTILE (not per slot tile): partitions
        # outside a subtile hold sentinel col=-1/norm=0, so every DVE sel
        # build and PE matmul runs on all 128 partitions (HW rejects
        # partition-offset PE operands in practice)
        self.SUBT = [self.bucket_subtiles(b_) for b_ in range(self.NBG)]
        self.NSUB_B = [len(s) for s in self.SUBT]       # subtiles per bucket
        self.SUBOFF = np.concatenate([[0], np.cumsum(self.NSUB_B)])
        self.SUB_PER_CALL = int(self.SUBOFF[-1])
        self.NSUB = self.NCALLS * self.SUB_PER_CALL     # sel columns total

    def bucket_subtiles(self, b_):
        """128-partition subtiles of bucket b_ (call-local): [(q0, kk)]."""
        out = []
        q = b_ * self.B_RUN
        end = q + self.B_RUN
        while q < end:
            if q % P:
                kk = min(P - q % P, end - q)
            else:
                kk = min(P, end - q)
            out.append((q, kk))
            q += kk
        return out


CFG = Cfg(N_NODES, DIM, N_CORES)


def rho(cfg, n):
    """node id -> permuted h_perm row (vectorized)."""
    s = n // cfg.SLICE
    m = n - s * cfg.SLICE
    blk = m // P
    p = m - blk * P
    half = blk // cfg.HALF
    jj = blk - half * cfg.HALF
    return s * cfg.SHARD + half * cfg.HS + p * cfg.HALF + jj


# ----------------------------------------------------------------------------
# host preprocessing
# ----------------------------------------------------------------------------

def host_prep(cfg, x, weight, edge_index):
    x = np.asarray(x, dtype=np.float32)
    weight = np.asarray(weight, dtype=np.float32)
    ei = np.asarray(edge_index)
    row = ei[0].astype(np.int64)
    col = ei[1].astype(np.int64)

    # deg counts outgoing (row) edges plus the implicit self-loop
    deg = (np.bincount(row, minlength=cfg.N) + 1).astype(np.float32)
    dis = (1.0 / np.sqrt(deg)).astype(np.float32)
    norm_e = (dis[row] * dis[col]).astype(np.float32)

    k = np.minimum(col // cfg.SHARD, cfg.NC - 1)
    dcol = col - k * cfg.SHARD
    blk = dcol // P
    col_local = dcol - blk * P
    g = blk // cfg.NBG
    b_ = blk - g * cfg.NBG
    s = row // cfg.SLICE
    c = np.minimum(s // cfg.CH_SL, cfg.NCH - 1)
    prow = rho(cfg, row)
    idxrel = prow - np.asarray(cfg.crow)[c]

    run = (c * cfg.NGRP + g) * cfg.NBG + b_      # run index within core
    nruns = cfg.NCALLS * cfg.NBG
    key = k * nruns + run
    order = np.lexsort((idxrel, key))            # by bucket, then address
    key_s = key[order]
    counts = np.bincount(key_s, minlength=cfg.NC * nruns)
    starts = np.concatenate([[0], np.cumsum(counts)])
    pos = np.arange(key_s.size) - starts[key_s]

    ok = pos < cfg.B_RUN
    run_s = run[order]
    slot = (run_s // cfg.NBG) * cfg.CALL + (run_s % cfg.NBG) * cfg.B_RUN + pos
    kk_s = k[order]

    idx_flat = np.zeros((cfg.NC, cfg.TOT), dtype=np.int16)
    o = order[ok]
    idx_flat[kk_s[ok], slot[ok]] = idxrel[o].astype(np.int16)

    # sel data packed per subtile: partitions outside the subtile keep the
    # sentinel (col=-1, norm=0) so full-128-partition sel builds are correct
    colv_p = np.full((cfg.NC, P, cfg.NSUB), -1.0, dtype=np.float32)
    normv_p = np.zeros((cfg.NC, P, cfg.NSUB), dtype=np.float32)
    b__s = run_s % cfg.NBG
    j_s = run_s // cfg.NBG
    cl_s = col_local[order].astype(np.float32)
    nm_s = norm_e[order]
    for b in range(cfg.NBG):
        cum = np.cumsum([0] + [kk for _, kk in cfg.SUBT[b]])
        m = ok & (b__s == b)
        si = np.searchsorted(cum, pos[m], side='right') - 1
        colc = j_s[m] * cfg.SUB_PER_CALL + cfg.SUBOFF[b] + si
        partn = (b * cfg.B_RUN + pos[m]) % P
        colv_p[kk_s[m], partn, colc] = cl_s[m]
        normv_p[kk_s[m], partn, colc] = nm_s[m]
    colv_p = np.ascontiguousarray(colv_p)
    normv_p = np.ascontiguousarray(normv_p)

    # overflow edges -> host correction (expected: a few hundred)
    ov = order[~ok]

    # per-call 16-partition wrapping of indices, replicated to 128 partitions
    idxw = idx_flat.reshape(cfg.NC, cfg.NCALLS, cfg.CW, 16)
    idxw = idxw.transpose(0, 3, 1, 2).reshape(cfg.NC, 16, cfg.IDXW)
    idxv_p = np.ascontiguousarray(np.tile(idxw, (1, 8, 1)))

    # self-loop scale dis[dest]^2, laid out [NC, 128, NBLK]
    nself = np.zeros((cfg.NC, cfg.SHARD), dtype=np.float32)
    ids = np.arange(cfg.SHARD)
    for core in range(cfg.NC):
        nd = min(cfg.SHARD, cfg.N - core * cfg.SHARD)
        d = dis[core * cfg.SHARD + ids[:nd]]
        nself[core, :nd] = d * d
    nself = np.ascontiguousarray(
        nself.reshape(cfg.NC, cfg.NBLK, P).transpose(0, 2, 1))

    xt = np.ascontiguousarray(x.T)
    xt_own = np.zeros((cfg.NC, cfg.DIM, cfg.SHARD), dtype=np.float32)
    for core in range(cfg.NC):
        nd = min(cfg.SHARD, cfg.N - core * cfg.SHARD)
        xt_own[core, :, :nd] = xt[:, core * cfg.SHARD:core * cfg.SHARD + nd]
    iota = np.broadcast_to(np.arange(P, dtype=np.float32), (P, P)).copy()

    in_maps = []
    for core in range(cfg.NC):
        in_maps.append({
            "xt": xt,
            "w": weight,
            "iota": iota,
            "colv": colv_p[core],
            "normv": normv_p[core],
            "nself": nself[core],
            "idx": idxv_p[core],
            "xt_own": xt_own[core],
        })

    # host correction for overflowed edges
    corr = None
    if ov.size:
        r, cdst = row[ov], col[ov]
        hsrc = x[r] @ weight
        m = hsrc * norm_e[ov][:, None]
        corr = np.zeros((cfg.N, cfg.DIM), dtype=np.float32)
        np.add.at(corr, cdst, m)
    return in_maps, corr


def unshard(cfg, outs, corr):
    out = np.empty((cfg.N, cfg.DIM), dtype=np.float32)
    for core in range(cfg.NC):
        o = outs[core]["outp"].reshape(P, cfg.NBLK, cfg.DIM)
        o = o.transpose(1, 0, 2).reshape(cfg.NBLK * P, cfg.DIM)
        nd = min(cfg.SHARD, cfg.N - core * cfg.SHARD)
        out[core * cfg.SHARD:core * cfg.SHARD + nd] = o[:nd]
    if corr is not None:
        out += corr
    return out


# ----------------------------------------------------------------------------
# device program
# ----------------------------------------------------------------------------

_PROG_CACHE = {}

PSB = 8  # h blocks batched per phase-0/1 psum bank


def build_program(cfg, reps=1, phases="F", queue_map=None):
    import contextlib

    import concourse.bass as bass  # noqa: F401
    import concourse.tile as tile
    from concourse import bacc, mybir

    f32 = mybir.dt.float32
    nc = bacc.Bacc("TRN2", target_bir_lowering=False, debug=False,
                   num_devices=cfg.NC, num_swdge_queues=4)

    xt = nc.dram_tensor("xt", [cfg.DIM, cfg.N], f32, kind="ExternalInput")
    w = nc.dram_tensor("w", [cfg.DIM, cfg.DIM], f32, kind="ExternalInput")
    iota = nc.dram_tensor("iota", [P, P], f32, kind="ExternalInput")
    colv = nc.dram_tensor("colv", [P, cfg.NSUB], f32, kind="ExternalInput")
    normv = nc.dram_tensor("normv", [P, cfg.NSUB], f32,
                           kind="ExternalInput")
    nself = nc.dram_tensor("nself", [P, cfg.NBLK], f32, kind="ExternalInput")
    idx = nc.dram_tensor("idx", [P, cfg.IDXW], mybir.dt.int16,
                         kind="ExternalInput")
    xt_own = nc.dram_tensor("xt_own", [cfg.DIM, cfg.SHARD], f32,
                            kind="ExternalInput")
    outp = nc.dram_tensor("outp", [P, cfg.NBLK * cfg.DIM], f32,
                          kind="ExternalOutput")
    h_perm = nc.dram_tensor("h_perm", [cfg.H_ROWS, cfg.DIM], f32)

    nc._gather_insts = []
    NB2 = math.ceil(cfg.HALF / PSB)  # psum batches per half-slice

    def halfslice_h(xp, pp1, w_sb, src_ap, nv, dst_sb, dst_col0):
        """h for one half-slice: matmul src (<=HS nodes) into dst_sb cols."""
        xs = xp.tile([cfg.DIM, cfg.HS], f32, tag="xs")
        if nv > 0:
            nc.sync.dma_start(out=xs[:, :nv], in_=src_ap)
        if nv < cfg.HS:
            nc.vector.memset(xs[:, nv:], 0)
        for m in range(NB2):
            jn = min(PSB, cfg.HALF - m * PSB)
            ps = pp1.tile([P, PSB * cfg.DIM], f32)
            for j8 in range(jn):
                q = m * PSB + j8
                nc.tensor.matmul(
                    out=ps[:, j8 * cfg.DIM:(j8 + 1) * cfg.DIM],
                    lhsT=xs[:, q * P:(q + 1) * P],
                    rhs=w_sb[:], start=True, stop=True)
            c0 = dst_col0 + m * PSB * cfg.DIM
            nc.vector.tensor_copy(out=dst_sb[:, c0:c0 + jn * cfg.DIM],
                                  in_=ps[:, :jn * cfg.DIM])

    with tile.TileContext(nc) as tc:
      with (tc.For_i(0, reps, 1) if reps > 1 else contextlib.nullcontext()):
        with tc.tile_pool(name="cp", bufs=1) as cp, \
             tc.tile_pool(name="gp", bufs=4) as gp, \
             tc.tile_pool(name="selp", bufs=6) as selp, \
             tc.tile_pool(name="xp", bufs=2) as xp, \
             tc.tile_pool(name="hp", bufs=2) as hp, \
             tc.tile_pool(name="pp1", bufs=3, space="PSUM") as pp1, \
             tc.tile_pool(name="pp2", bufs=4, space="PSUM") as pp2:
            w_sb = cp.tile([cfg.DIM, cfg.DIM], f32)
            nc.sync.dma_start(out=w_sb[:], in_=w[:])
            iota_sb = cp.tile([P, P], f32)
            nc.sync.dma_start(out=iota_sb[:], in_=iota[:])
            colv_sb = cp.tile([P, cfg.NSUB], f32)
            nc.sync.dma_start(out=colv_sb[:], in_=colv[:])
            normv_sb = cp.tile([P, cfg.NSUB], f32)
            nc.sync.dma_start(out=normv_sb[:], in_=normv[:])
            nself_sb = cp.tile([P, cfg.NBLK], f32)
            nc.sync.dma_start(out=nself_sb[:], in_=nself[:])
            idx_sb = cp.tile([P, cfg.IDXW], mybir.dt.int16)
            nc.sync.dma_start(out=idx_sb[:], in_=idx[:])
            out_sb = cp.tile([P, cfg.NBLK * cfg.DIM], f32)
            hs_own = cp.tile([P, cfg.NBLK * cfg.DIM], f32)

            # ---- phase 0: hs_own = (x_own @ W) for the self-loop term ----
            for half in range(2):
                n0 = half * cfg.HS
                halfslice_h(xp, pp1, w_sb, xt_own[:, n0:n0 + cfg.HS], cfg.HS,
                            hs_own, n0 // P * cfg.DIM)

            def emit_slices(slices):
                for s2 in slices:
                    for half in range(2):
                        n0 = s2 * cfg.SLICE + half * cfg.HS
                        nv = max(0, min(cfg.HS, cfg.N - n0))
                        hst = hp.tile([P, cfg.HALF * cfg.DIM], f32, tag="hst")
                        halfslice_h(xp, pp1, w_sb,
                                    xt[:, n0:n0 + nv] if nv else None,
                                    nv, hst, 0)
                        base = s2 * cfg.SHARD + half * cfg.HS
                        dst = h_perm[base:base + cfg.HS, :]
                        dst = dst.rearrange("(p j) d -> p (j d)", p=P)
                        nc.sync.dma_start(out=dst, in_=hst[:])

            if "N" in phases:       # no interleave: all h slices upfront
                emit_slices(range(2 * cfg.NCH))

            # ---- main loop: per source chunk, h slices then gathers ------
            for c in range(cfg.NCH):
                if "N" not in phases:
                    emit_slices((2 * c, 2 * c + 1))

                # gathers + scatter matmuls for this chunk
                for g in range(cfg.NGRP):
                    j = c * cfg.NGRP + g
                    gb = gp.tile([P, cfg.CT, cfg.DIM], f32, tag="gbuf")
                    gi = nc.gpsimd.dma_gather(
                        out_ap=gb[:],
                        in_ap=h_perm[cfg.crow[c]:cfg.crow[c + 1], :],
                        idxs_ap=idx_sb[:, j * cfg.CW:(j + 1) * cfg.CW],
                        num_idxs=cfg.CALL,
                        num_idxs_reg=cfg.CALL,
                        elem_size=cfg.DIM,
                        single_packet=False,
                        queue_num=(queue_map or {}).get(j, 0),
                    )
                    nc._gather_insts.append((j, gi.ins.name))
                    if "G" in phases:
                        nc.vector.tensor_copy(out=out_sb[:, :cfg.DIM],
                                              in_=gb[:, 0, :])
                        continue

                    subts = [(b_, sk, q0) for b_ in range(cfg.NBG)
                             for sk, (q0, _) in enumerate(cfg.SUBT[b_])]
                    ps = pp2.tile([P, cfg.NBG * cfg.DIM], f32)
                    for si, (b_, sk, q0) in enumerate(subts):
                        t = q0 // P
                        T = (j * cfg.SUB_PER_CALL + int(cfg.SUBOFF[b_])
                             + sk)
                        sel = selp.tile([P, P], f32)
                        nc.vector.tensor_scalar(
                            out=sel[:],
                            in0=iota_sb[:],
                            scalar1=colv_sb[:, T:T + 1],
                            scalar2=normv_sb[:, T:T + 1],
                            op0=mybir.AluOpType.is_equal,
                            op1=mybir.AluOpType.mult)
                        nc.tensor.matmul(
                            out=ps[:, b_ * cfg.DIM:(b_ + 1) * cfg.DIM],
                            lhsT=sel[:],
                            rhs=gb[:, t, :],
                            start=(si == 0), stop=(si == len(subts) - 1),
                            skip_group_check=True)

                    gc = slice(g * cfg.NBG * cfg.DIM,
                               (g + 1) * cfg.NBG * cfg.DIM)
                    if c == 0 or "A" in phases:
                        nc.vector.tensor_copy(out=out_sb[:, gc], in_=ps[:])
                    else:
                        nc.vector.tensor_add(out_sb[:, gc], out_sb[:, gc],
                                             ps[:])
                    if "S" in phases:
                        continue
                    if c == cfg.NCH - 1:
                        # self-loop: out += dis[dest]^2 * hs_own, per block
                        for b_ in range(cfg.NBG):
                            b = g * cfg.NBG + b_
                            bc = slice(b * cfg.DIM, (b + 1) * cfg.DIM)
                            nc.vector.scalar_tensor_tensor(
                                out=out_sb[:, bc],
                                in0=hs_own[:, bc],
                                scalar=nself_sb[:, b:b + 1],
                                in1=out_sb[:, bc],
                                op0=mybir.AluOpType.mult,
                                op1=mybir.AluOpType.add)

            nc.sync.dma_start(out=outp[:], in_=out_sb[:])

    nc.compile()
    return nc


def gather_queue_map(nc):
    """call_j -> queue: DMASW lane % 4, except lanes also used by plain
    Pool DMAs (which are implicitly queue 0) are pinned to queue 0."""
    from concourse.tile_sem_assignment import PROC_NAME_TO_IDX
    idx2name = {v: k for k, v in PROC_NAME_TO_IDX.items()}
    gather_names = {name for _, name in nc._gather_insts}
    locked = set()
    for name, inst in nc.inst_map.items():
        proc = idx2name.get(getattr(inst, "bass_scheduled_proc", None), "")
        if (proc.startswith("DMASW") and name not in gather_names):
            locked.add(proc)
    qm = {}
    for j, name in nc._gather_insts:
        inst = nc.inst_map[name]
        proc = idx2name[inst.bass_scheduled_proc]
        assert proc.startswith("DMASW")
        qm[j] = 0 if proc in locked else int(proc[5:]) % 4
    return qm


def build_with_queues(cfg, reps=1, phases="F", rotate=True):
    if not rotate:
        return build_program(cfg, reps=reps, phases=phases, queue_map=None)
    qm = {}
    for _ in range(3):
        nc = build_program(cfg, reps=reps, phases=phases, queue_map=qm)
        qm2 = gather_queue_map(nc)
        if qm2 == qm:
            return nc
        qm = qm2
    return build_program(cfg, reps=reps, phases=phases, queue_map=None)


def get_program(cfg):
    key = (cfg.N, cfg.DIM, cfg.NC, cfg.SLICE, cfg.B_RUN, cfg.NBG, cfg.CH_SL)
    if key not in _PROG_CACHE:
        _PROG_CACHE[key] = build_with_queues(cfg)
    return _PROG_CACHE[key]


# ----------------------------------------------------------------------------
# entry point
# ----------------------------------------------------------------------------

def kernel(x, weight, edge_index):
    from concourse.bass_utils import run_bass_kernel_spmd

    cfg = CFG
    in_maps, corr = host_prep(cfg, x, weight, edge_index)
    nc = get_program(cfg)
    res = run_bass_kernel_spmd(nc, in_maps, list(range(cfg.NC)))
    return unshard(cfg, res.results, corr)
